# revision 7
# baseline (speedup 1.0000x reference)
"""MergedEmbeddingBag kernel for 8 TRN2 NeuronCores.

Strategy (batch-sharded SPMD + per-table-pair compaction + dma_gather):
  - Global work: T=26 tables x B=4096 bags of L=10 lookups each into
    [V=50000, D=128] f32 tables, sum-pooled, concat with dense.
  - Batch sharding: core m handles bags [m*512, (m+1)*512) of EVERY
    table -> 26*512 = 13312 bags/core, perfectly uniform SPMD.
  - The fast gather path is the Q7 `dma_gather` extended instruction
    (vectorized descriptor generation), whose indices are int16.  To fit
    int16, the host compacts weights per (core, table-pair): the <=10240
    distinct rows referenced by one core in tables (2s, 2s+1) are packed
    into slot s of a [13*10240, 128] per-core weight buffer, and the
    lookup indices are remapped to compacted ids (< 10240).
  - Per core: 13 dma_gather calls of 10240 rows (one per table pair),
    in-place DVE add tree pools the L=10 rows of each bag, one strided
    store per call.  The dense column block is passed through on host.

dma_gather HW contract (probed on silicon):
  - stream position i reads its int16 index from idxs tile partition
    16 + (i%16), word i//16 (queue 0).  (The CoreSim reads partitions
    0..15, so indices are duplicated into both ranges.)
  - gathered row i lands in dst partition i%128, free slot i//128.
"""

import numpy as np

import concourse.bacc as bacc
import concourse.bass as bass
import concourse.mybir as mybir
import concourse.tile as tile
from concourse.bass_utils import run_bass_kernel_spmd

T, B, L, V, D = 26, 4096, 10, 50000, 128
M = 8                          # cores
BPC = T * B // M               # 13312 bags per core
BAGS_PER_TABLE = B // M        # 512
PAIRS = T // 2                 # 13 table pairs == calls per core
BAGS_PER_CALL = 2 * BAGS_PER_TABLE  # 1024
NIDX = BAGS_PER_CALL * L       # 10240 gathered rows per call
CAP = NIDX                     # compacted rows capacity per pair slot
W_ROWS = PAIRS * CAP           # 133120
IDXW = NIDX // 16              # 640 idx words per channel per call

_CACHE = {}


def _build_nc(repeats=1):
    key = ("nc", repeats)
    if key in _CACHE:
        return _CACHE[key]
    nc = bacc.Bacc("TRN2", target_bir_lowering=False, debug=False, num_devices=M)
    w = nc.dram_tensor("w", [W_ROWS, D], mybir.dt.float32, kind="ExternalInput").ap()
    idx = nc.dram_tensor(
        "idx", [128, PAIRS * IDXW], mybir.dt.int16, kind="ExternalInput"
    ).ap()
    out = nc.dram_tensor("out", [BPC, D], mybir.dt.float32, kind="ExternalOutput").ap()
    # out row (c*1024 + p*8 + j) <- pooled[p, j*128:(j+1)*128] of call c
    out_v = out.rearrange("(c p j) d -> c p (j d)", c=PAIRS, p=128, j=8)

    BLK = 8 * D  # 1024 elems = one l-block (8 bags x 128)

    NSUB = NIDX // 128 // 8  # 10 sub-gathers per pair (one per bag element l)
    nidx = NIDX // NSUB  # 1024 rows per sub-gather
    with tile.TileContext(nc) as tc:
        with (
            tc.tile_pool(name="idxp", bufs=1) as idxp,
            tc.tile_pool(name="gathp", bufs=2) as gathp,
        ):
            idx_sb = idxp.tile([128, PAIRS * IDXW], mybir.dt.int16)
            nc.sync.dma_start(out=idx_sb[:], in_=idx[:])
            for c in [c for _ in range(repeats) for c in range(PAIRS)]:
                # sub-gather l covers stream positions [l*1024, (l+1)*1024) of
                # the pair's index list == l-block l for all (p, j); idx slice
                # columns line up because 1024 % 16 == 0.
                tiles = []
                for g in range(NSUB):
                    gt = gathp.tile([128, BLK], mybir.dt.float32, tag=f"g{g}")
                    nc.gpsimd.dma_gather(
                        out_ap=gt[:].rearrange("p (k d) -> p k d", d=D),
                        in_ap=w[c * CAP : (c + 1) * CAP, :],
                        idxs_ap=idx_sb[
                            :,
                            c * IDXW + g * (nidx // 16) : c * IDXW
                            + (g + 1) * (nidx // 16),
                        ],
                        num_idxs=nidx,
                        num_idxs_reg=nidx,
                        elem_size=D,
                        single_packet=True,
                    )
                    tiles.append(gt)
                # add tree across the 10 per-l tiles (each [128, 8*128])
                for g in range(5):
                    nc.vector.tensor_add(
                        out=tiles[g][:], in0=tiles[g][:], in1=tiles[g + 5][:]
                    )
                nc.vector.tensor_add(
                    out=tiles[0][:], in0=tiles[0][:], in1=tiles[2][:]
                )
                nc.vector.tensor_add(
                    out=tiles[1][:], in0=tiles[1][:], in1=tiles[3][:]
                )
                nc.vector.tensor_add(
                    out=tiles[0][:], in0=tiles[0][:], in1=tiles[1][:]
                )
                nc.vector.tensor_add(
                    out=tiles[0][:], in0=tiles[0][:], in1=tiles[4][:]
                )
                nc.sync.dma_start(out=out_v[c], in_=tiles[0][:])
    nc.compile()
    _CACHE[key] = nc
    return nc


def _prep_inputs(index, weights):
    """Per-core inputs: compacted pair-wise weights + snake-laid int16 ids."""
    index = np.asarray(index)
    w_flat = np.asarray(weights, dtype=np.float32).reshape(T * V, D)
    in_maps = []
    for m in range(M):
        # per-table slice of this core's 512 bags -> [T, 5120]
        idx_m = index[:, m * BAGS_PER_TABLE * L : (m + 1) * BAGS_PER_TABLE * L]
        w_core = np.zeros((W_ROWS, D), np.float32)
        idx_core = np.zeros((128, PAIRS * IDXW), np.int16)
        for s in range(PAIRS):
            # local row key within the pair: [0, 2V)
            keys = np.concatenate(
                [idx_m[2 * s], idx_m[2 * s + 1] + V]
            )  # [10240] order: table 2s bags, then 2s+1 bags
            uniq, inv = np.unique(keys, return_inverse=True)
            u = len(uniq)
            assert u <= CAP
            w_core[s * CAP : s * CAP + u] = w_flat[2 * s * V + uniq]
            # arr[q, l]: compact id of element l of call-local bag q
            arr = inv.reshape(BAGS_PER_CALL, L)
            # stream position i = (l*8+j)*128 + p for bag q = p*8+j
            lst = (
                arr.reshape(128, 8, L).transpose(2, 1, 0).reshape(NIDX).astype(np.int16)
            )
            # snake: stream[i] read from partition 16+(i%16) (HW) / (i%16) (sim)
            snake = lst.reshape(IDXW, 16).T  # [16, IDXW]
            idx_core[0:16, s * IDXW : (s + 1) * IDXW] = snake
            idx_core[16:32, s * IDXW : (s + 1) * IDXW] = snake
        in_maps.append({"w": w_core, "idx": idx_core})
    return in_maps


def kernel(index, offsets, dense, weights):
    nc = _build_nc()
    in_maps = _prep_inputs(index, weights)
    res = run_bass_kernel_spmd(nc, in_maps, core_ids=list(range(M))).results
    # res[m]["out"][i_loc] = pooled(t=i_loc//512, b=m*512 + i_loc%512)
    pooled = np.empty((T, B, D), np.float32)
    for m in range(M):
        pooled[:, m * BAGS_PER_TABLE : (m + 1) * BAGS_PER_TABLE] = res[m][
            "out"
        ].reshape(T, BAGS_PER_TABLE, D)
    out = np.empty((B, (T + 1) * D), np.float32)
    out[:, :D] = np.asarray(dense, dtype=np.float32)
    out[:, D:] = pooled.transpose(1, 0, 2).reshape(B, T * D)
    return out
